# revision 6
# baseline (speedup 1.0000x reference)
"""Trainium2 Bass kernel for nn_Attention_82540681494971.

Spatial self-attention block (LDM AttnBlock style, unscaled):
  qkv = conv1x1(x);  s = q^T k  [n x n] per (b,head);  attn = softmax(s, axis=-1)
  out[d,m] = sum_n v[d,n] attn[n,m];  y = conv1x1(out)

Shapes: B=4, C=64, H=W=64 -> n=4096 tokens, HEAD=4, d=16.

Sharding: 8 cores, core c handles batch b=c//2 and heads (0,1) if c%2==0
else (2,3). Each core computes a partial projection output over its two
heads' channels; host sums the two partials per batch and adds proj bias.

Key algebra: attn[n,m] = E[n,m]/rowsum[n] with E=exp(s). Fold the
projection INTO the AV contraction: per n-tile t,
  Wn_t[n, o] = rinv[n] * (v^T wp^T)[n, o]          (128 x 64, bf16)
  y[o, m]   += sum_t Wn_t^T @ E_t[:, m]            (PE chains, M=64)
so the per-head d=16 intermediate never materializes and the separate
projection pass disappears. AV chains alternate PE column strips
(tile_position (0,0)/(0,64)) so two chains stream concurrently; y
partial sums live in y_sb[128, 2048] (m-chunk mc at partition half
mc%2, col block mc//2).

Scores are exact-fp32-grade via a 3-term bf16 split (q=q_hi+q_lo,
k=k_hi+k_lo, drop lo*lo): s = [q_hi;q_lo;q_hi]^T [k_hi;k_hi;k_lo]
(K=48 stacked), mirrored on PE row strips 0/64 for 2-way streaming.

PSUM budget (8 banks): scores 5 (ring 1536+1024, chunks 1536/1024/1536),
AV 2 (two [128,512] chain buffers), Wn 1 ([128,64] JIT tiles).
"""

import numpy as np
from contextlib import ExitStack

import concourse.bass as bass
import concourse.mybir as mybir
import concourse.tile as tile
from concourse import bacc
from concourse.bass import ts, ds
from concourse.bass_utils import run_bass_kernel_spmd

F32 = mybir.dt.float32
BF16 = mybir.dt.bfloat16
AF = mybir.ActivationFunctionType

B, C, HEAD, D = 4, 64, 4, 16
N = 4096          # tokens = H*W
NT = 128          # n-tile (partition) size
NTILES = N // NT  # 32
MC = 512          # matmul free-dim chunk
MCN = N // MC     # 8 m-chunks
NCH = 4           # exp chunks per tile, 1024 wide, psum ring of 2x[128,1024]
CW = N // NCH     # 1024
GROUPS = [6, 6, 6, 6, 6, 2]   # AV supergroup sizes per head
NSG = len(GROUPS)
E_DT = BF16       # dtype of exp(s) for the AV matmul


def _body(tc, y, x1, wq, wk, wv, wp):
    nc = tc.nc
    ctx = ExitStack()
    with ctx:
        pp = ctx.enter_context(tc.tile_pool(name="persist", bufs=1))
        cp = ctx.enter_context(tc.tile_pool(name="consts", bufs=1))

        # ---- constants ----
        wq_t = cp.tile([C + 1, 2 * D], F32)
        wk_t = cp.tile([C + 1, 2 * D], F32)
        wv_t = cp.tile([C + 1, 2 * D], F32)
        wp_t = cp.tile([D, 2 * C], F32)      # [16, 128]: head0 cols 0-63, head1 64-127
        nc.sync.dma_start(wq_t[:], wq[:])
        nc.sync.dma_start(wk_t[:], wk[:])
        nc.sync.dma_start(wv_t[:], wv[:])
        nc.sync.dma_start(wp_t[:], wp[:])

        # ---- persistent SBUF ----
        # [q_hi; q_lo; q_hi] stacked on partitions 0-47 and mirrored on
        # 64-111 (second PE row strip). head-major cols.
        qsp = pp.tile([64 + 3 * D, 2 * N], BF16)
        ksp = pp.tile([64 + 3 * D, 2 * N], BF16)
        v_sb = pp.tile([D, 2 * N], F32)      # v in [d, n] layout, head-major
        # y partials: m-chunk mc at partitions 64*(mc%2), cols 512*(mc//2)
        y_sb = pp.tile([NT, N // 2], F32)

        # ---- phase 0: qkv + bf16 hi/lo split ----
        with (
            tc.tile_pool(name="x1p", bufs=1) as xp,
            tc.tile_pool(name="qkf", bufs=2) as qf,
            tc.tile_pool(name="p0psum", bufs=3, space="PSUM") as p0,
        ):
            x1_t = xp.tile([C + 1, N], F32)
            for i in range(8):  # parallel DMA queues
                nc.sync.dma_start(x1_t[:, ts(i, N // 8)], x1[:, ts(i, N // 8)])

            for which, w_t, dst, dup in (
                ("q", wq_t, qsp, 2),   # dup row-block 2 gets hi
                ("k", wk_t, ksp, 1),   # dup row-block 1 gets hi
            ):
                hi_t = qf.tile([D, 2 * N], BF16, tag="hi", name=f"hi_{which}")
                lo_t = qf.tile([D, 2 * N], BF16, tag="lo", name=f"lo_{which}")
                for h in range(2):
                    for mc in range(N // MC):
                        ps = p0.tile([D, MC], F32, tag="p0")
                        nc.tensor.matmul(
                            ps[:], w_t[:, ts(h, D)], x1_t[:, ts(mc, MC)],
                            start=True, stop=True)
                        sl = ds(h * N + mc * MC, MC)
                        # hi-cast on ACT (idle in phase 0), lo-sub on DVE
                        nc.scalar.copy(hi_t[:, sl], ps[:])
                        nc.vector.tensor_sub(lo_t[:, sl], ps[:], hi_t[:, sl])
                # assemble K=48 stack via SBUF->SBUF DMA (no partition
                # alignment limits on DMA); chunked for queue parallelism
                lo_block = 1 if which == "q" else 2
                for i in range(4):
                    sl = ts(i, N // 2)
                    for b0 in (0, 64):
                        nc.sync.dma_start(dst[ds(b0, D), sl], hi_t[:, sl])
                        nc.sync.dma_start(
                            dst[ds(b0 + lo_block * D, D), sl], lo_t[:, sl])
                        nc.sync.dma_start(
                            dst[ds(b0 + dup * D, D), sl], hi_t[:, sl])

            # v in [d, n] layout: one matmul per (head, chunk)
            for h in range(2):
                for mc in range(N // MC):
                    psv = p0.tile([D, MC], F32, tag="p0")
                    nc.tensor.matmul(
                        psv[:], wv_t[:, ts(h, D)], x1_t[:, ts(mc, MC)],
                        start=True, stop=True)
                    dst_ap = v_sb[:, ds(h * N + mc * MC, MC)]
                    if mc % 2 == 0:
                        nc.vector.tensor_copy(dst_ap, psv[:])
                    else:
                        nc.scalar.copy(dst_ap, psv[:])

        # ---- phase 1: attention, software-pipelined ----
        # Per tile: scores (3 psum chunks, strip-alternating matmuls) ->
        # exp with row-sum accum -> rinv -> JIT Wn = (v^T wp)*rinv ->
        # (next supergroup) AV chains of the previous supergroup woven
        # between score chunks so the PE never idles and ACT stays fed.
        with (
            tc.tile_pool(name="ep", bufs=14) as ep,
            tc.tile_pool(name="rp", bufs=4) as rp,
            tc.tile_pool(name="wnp", bufs=14) as wnp,
            tc.tile_pool(name="sapsum", bufs=2, space="PSUM") as sp,
            tc.tile_pool(name="avpsum", bufs=3, space="PSUM") as ap,
            tc.tile_pool(name="wnpsum", bufs=1, space="PSUM") as wp_ps,
        ):
            def av_chain(sg_idx, chains, mc):
                """Emit the AV chain for m-chunk mc of supergroup sg_idx
                (list of (wn_tile, e_tile)), then its y_sb evac."""
                strip = 64 * (mc % 2)
                blk = ts(mc // 2, MC)
                yps = ap.tile([NT, MC], F32, tag="av", name=f"av{sg_idx}_{mc}")
                out_ap = yps[ds(strip, C), :]
                gl = len(chains)
                for j, (wn_t, e_t) in enumerate(chains):
                    nc.tensor.matmul(
                        out_ap, wn_t[:], e_t[:, ts(mc, MC)],
                        start=(j == 0), stop=(j == gl - 1),
                        tile_position=(0, strip))
                dst = y_sb[ds(strip, C), blk]
                if sg_idx == 0:
                    nc.vector.tensor_copy(dst, out_ap)
                else:
                    nc.vector.tensor_add(dst, dst, out_ap)
                if sg_idx == 2 * NSG - 1:  # final supergroup: stream out
                    nc.sync.dma_start(y[:, ts(mc, MC)], dst)

            prev = None    # list of (wn_tile, e_tile) for previous supergroup
            prev_idx = -1
            pend = []      # m-chunks of prev still to weave
            cur = []
            sgi = 0
            slot = 0
            for h in range(2):
                gleft = list(GROUPS)
                for nt in range(NTILES):
                    e_t = ep.tile([NT, N], E_DT, tag="e", name=f"e{h}_{nt}")
                    rsp = rp.tile([NT, NCH], F32, tag="rs", name="rsp")
                    for ci in range(NCH):
                        s_ps = sp.tile([NT, CW], F32, tag="sa", name="s_ps")
                        for i in range(CW // MC):
                            b0 = 64 * ((ci * (CW // MC) + i) % 2)
                            nc.tensor.matmul(
                                s_ps[:, ts(i, MC)],
                                qsp[ds(b0, 3 * D), ds(h * N + nt * NT, NT)],
                                ksp[ds(b0, 3 * D),
                                    ds(h * N + ci * CW + i * MC, MC)],
                                start=True, stop=True, tile_position=(b0, 0))
                        nc.scalar.activation(
                            e_t[:, ts(ci, CW)], s_ps[:],
                            AF.Exp, accum_out=rsp[:, ds(ci, 1)])
                        # weave prev-supergroup AV chains, 1 per 3 slots
                        if pend and slot % 3 == 1:
                            av_chain(prev_idx, prev, pend.pop(0))
                        slot += 1
                    rinv = rp.tile([NT, 1], F32, tag="ri", name="rinv")
                    rs = rp.tile([NT, 1], F32, tag="r1", name="rs")
                    nc.vector.reduce_sum(
                        rs[:], rsp[:], axis=mybir.AxisListType.X)
                    nc.vector.reciprocal(rinv[:], rs[:])
                    # JIT Wn: (v^T @ wp) * rinv -> bf16  [128 n, 64 o]
                    wn_ps = wp_ps.tile([NT, C], F32, tag="wn", name="wn_ps")
                    nc.tensor.matmul(
                        wn_ps[:], v_sb[:, ds(h * N + nt * NT, NT)],
                        wp_t[:, ts(h, C)], start=True, stop=True)
                    wn_t = wnp.tile([NT, C], BF16, tag="wn",
                                    name=f"wn{h}_{nt}")
                    nc.vector.tensor_scalar_mul(wn_t[:], wn_ps[:], rinv[:])
                    cur.append((wn_t, e_t))
                    if len(cur) == gleft[0]:
                        # flush any unwoven chains of the previous supergroup
                        while pend:
                            av_chain(prev_idx, prev, pend.pop(0))
                        prev, cur = cur, []
                        gleft.pop(0)
                        prev_idx = sgi
                        sgi += 1
                        pend = list(range(MCN))

            # ---- tail: flush the last supergroup's chains ----
            while pend:
                av_chain(prev_idx, prev, pend.pop(0))


_PROGRAM = None


def _get_program():
    global _PROGRAM
    if _PROGRAM is None:
        nc = bacc.Bacc("TRN2", target_bir_lowering=False, debug=False,
                       num_devices=8)
        x1 = nc.dram_tensor("x1", [C + 1, N], F32, kind="ExternalInput").ap()
        wq = nc.dram_tensor("wq", [C + 1, 2 * D], F32, kind="ExternalInput").ap()
        wk = nc.dram_tensor("wk", [C + 1, 2 * D], F32, kind="ExternalInput").ap()
        wv = nc.dram_tensor("wv", [C + 1, 2 * D], F32, kind="ExternalInput").ap()
        wp = nc.dram_tensor("wp", [D, 2 * C], F32, kind="ExternalInput").ap()
        y = nc.dram_tensor("y", [C, N], F32, kind="ExternalOutput").ap()
        with tile.TileContext(nc) as tc:
            _body(tc, y, x1, wq, wk, wv, wp)
        nc.compile()
        _PROGRAM = nc
    return _PROGRAM


def _make_in_maps(x, qkv_w, qkv_b, proj_w, proj_b=None):
    x = np.asarray(x, dtype=np.float32)
    qkv_w = np.asarray(qkv_w, dtype=np.float32)
    qkv_b = np.asarray(qkv_b, dtype=np.float32)
    proj_w = np.asarray(proj_w, dtype=np.float32)

    in_maps = []
    for core in range(8):
        b = core // 2
        h0 = 2 * (core % 2)
        heads = (h0, h0 + 1)
        x1 = np.concatenate(
            [x[b].reshape(C, N), np.ones((1, N), np.float32)], axis=0)

        def aug_qk(block):
            w = np.empty((C + 1, 2 * D), np.float32)
            for j, h in enumerate(heads):
                rows = slice(block * C + h * D, block * C + (h + 1) * D)
                w[:C, j * D:(j + 1) * D] = qkv_w[rows, :].T
                w[C, j * D:(j + 1) * D] = qkv_b[rows]
            return w

        wp = np.concatenate(
            [np.ascontiguousarray(proj_w[:, h * D:(h + 1) * D].T)
             for h in heads], axis=1)  # [16, 128]

        in_maps.append({
            "x1": np.ascontiguousarray(x1),
            "wq": aug_qk(0),
            "wk": aug_qk(1),
            "wv": aug_qk(2),
            "wp": np.ascontiguousarray(wp),
        })
    return in_maps


def run_cores(inputs, **kw):
    """Compile+run on the 8 cores; returns BassKernelResults."""
    nc = _get_program()
    in_maps = _make_in_maps(**inputs)
    return run_bass_kernel_spmd(nc, in_maps, list(range(8)), **kw)


def kernel(x, qkv_w, qkv_b, proj_w, proj_b):
    res = run_cores(dict(x=x, qkv_w=qkv_w, qkv_b=qkv_b,
                         proj_w=proj_w, proj_b=proj_b))
    proj_b = np.asarray(proj_b, dtype=np.float32)
    parts = [r["y"] for r in res.results]
    out = np.empty((B, C, N), np.float32)
    for b in range(B):
        out[b] = parts[2 * b] + parts[2 * b + 1] + proj_b[:, None]
    return out.reshape(B, C, 64, 64)


if __name__ == "__main__":
    _get_program()
    print("program built OK")
